# revision 27
# baseline (speedup 1.0000x reference)
"""BERT self-attention on 8 Trainium2 NeuronCores.

Sharding: data-parallel over batch (batch=8, one element per core).

v2: projection/attention overlap. The baseline ran all of Q/K/V
projection (PE-bound, ~82us) before any attention, leaving the ACT
engine (exp, ~133us total) idle, then ran an ACT-bound attention phase.
v2 streams per feature-chunk: V projection first, then for each fc
(= 2 heads) project kt/qt and immediately run that fc's attention,
software-pipelined exactly like the baseline (pair i's PV interleaves
with pair i+1's scores; flush trails one pair). ACT exp now starts
~35us in and overlaps all remaining PE work.

Numerics: projections and scores in float32r (full PE rate at N=512);
exp output (es), V, and the context copies in bf16 (halves SBUF so all
phases can stay live; numerator and denominator share the same bf16 es
so the softmax division cancels most of the quantization). Output
normalize emits fp32.

Per-fc weight slices ([128,128] per k-chunk) are DMA'd double-buffered;
output blocks store per (pair, q-block) so the 4MB result DMA spreads
across the run.
"""

import contextlib
import sys
import time

if "/opt/trn_rl_repo" not in sys.path:
    sys.path.insert(0, "/opt/trn_rl_repo")

import numpy as np

import concourse.bacc as bacc
import concourse.mybir as mybir
from concourse import tile
from concourse.bass_utils import run_bass_kernel_spmd
from concourse.masks import make_identity

S = 1024          # seq len
H = 1024          # hidden
NH = 16           # heads
D = 64            # head dim
P = 128           # partitions
NQ = 512          # q free-dim chunk
KC = S // P       # 8 seq chunks of 128
QC = S // NQ      # 2 q chunks of 512
FC = H // P       # 8 feature chunks of 128
F32 = mybir.dt.float32
F32R = mybir.dt.float32r
BF16 = mybir.dt.bfloat16

_CACHE: dict = {}


def _build(loop: int = 1):
    """Build the per-core module. loop>1 wraps the whole body in a
    hardware For_i loop (timing amplification only)."""
    nc = bacc.Bacc("TRN2", target_bir_lowering=False, debug=False)

    # activations and weights arrive host-converted to bf16: halves input
    # DMA and SBUF, and every 128-column stationary operand gets Fast
    # Weight Load (2x LDWEIGHTS) on hardware
    xT = nc.dram_tensor("xT", [H, S], BF16, kind="ExternalInput")
    # wqR/wkR are host-rearranged to [FC*KC*P, P]: block (fc, k) holds
    # wT[k*128:(k+1)*128, fc*128:(fc+1)*128] contiguously, so each per-fc
    # weight slice is one contiguous DMA instead of a 512B-burst
    # strided read
    wqR = nc.dram_tensor("wqR", [FC * KC * P, P], BF16, kind="ExternalInput")
    wkR = nc.dram_tensor("wkR", [FC * KC * P, P], BF16, kind="ExternalInput")
    wvT = nc.dram_tensor("wvT", [H, H], BF16, kind="ExternalInput")
    bq = nc.dram_tensor("bq", [H], F32, kind="ExternalInput")
    bk = nc.dram_tensor("bk", [H], F32, kind="ExternalInput")
    bv = nc.dram_tensor("bv", [H], F32, kind="ExternalInput")
    mask = nc.dram_tensor("mask", [S], F32, kind="ExternalInput")
    out = nc.dram_tensor("out", [S, H], F32, kind="ExternalOutput")

    with tile.TileContext(nc) as tc:
        with (
            tc.For_i(0, loop, 1) if loop > 1 else contextlib.nullcontext(),
            tc.tile_pool(name="persist", bufs=1) as pp,
            # PSUM budget (16KB/partition): scores "mm" 2x6KB + pv/tp 2x2KB
            tc.tile_pool(name="ps512", bufs=2, space="PSUM") as psA,
            tc.tile_pool(name="pspv", bufs=2, space="PSUM") as psB,
        ):
            # ---- constants / small tiles ----
            ident = pp.tile([P, P], BF16, tag="ident")
            make_identity(nc, ident[:])

            ones1 = pp.tile([1, P], F32, tag="ones1")
            nc.gpsimd.memset(ones1[:], 1.0)
            ones16 = pp.tile([P, NH], F32, tag="ones16")
            nc.gpsimd.memset(ones16[:], 1.0)

            bv_row = pp.tile([1, H], F32, tag="bvrow")
            nc.sync.dma_start(bv_row[:], bv.ap().rearrange("(o h) -> o h", o=1))

            # partition-gather DMAs are descriptor-heavy; keep them on a
            # hwdge queue (scalar) but off the critical sync queue (not
            # needed until the first bias add / first exp, ~15us in)
            bq_sb = pp.tile([P, FC], F32, tag="bq")
            bk_sb = pp.tile([P, FC], F32, tag="bk")
            mask_sb = pp.tile([P, KC], F32, tag="mask")
            nc.scalar.dma_start(bq_sb[:], bq.ap().rearrange("(c p) -> p c", p=P))
            nc.scalar.dma_start(bk_sb[:], bk.ap().rearrange("(c p) -> p c", p=P))
            nc.scalar.dma_start(mask_sb[:], mask.ap().rearrange("(c p) -> p c", p=P))
            # the additive mask folds into V as a multiplicative per-key
            # factor: exp(s+m) = exp(s)*exp(m), and scaling P's key-column
            # equals scaling V's key-row -- including the ones column, so
            # the softmax denominator stays correct. This frees the exp
            # instructions from the per-k-chunk bias operand, letting one
            # activation span three 512-wide score segments.
            em_sb = pp.tile([P, KC], F32, tag="em")
            nc.scalar.activation(
                em_sb[:], mask_sb[:], mybir.ActivationFunctionType.Exp)
            bvb = pp.tile([P, H], F32, tag="bvb")
            for half in range(2):
                psb = psA.tile([P, NQ], F32, tag="mm")
                nc.tensor.matmul(
                    psb[:], ones1[:], bv_row[:, half * NQ:(half + 1) * NQ],
                    start=True, stop=True,
                )
                nc.vector.tensor_copy(bvb[:, half * NQ:(half + 1) * NQ], psb[:])

            # ---- persistent activations ----
            xt = [pp.tile([P, S], BF16, tag=f"xt{i}", name=f"xt{i}")
                  for i in range(KC)]
            for i in range(KC):
                # split the 4MB xT load across two queues so the first
                # projection group (which needs all 8 chunks) unblocks early
                eng = nc.sync if i < KC // 2 else nc.gpsimd
                eng.dma_start(xt[i][:], xT.ap()[i * P:(i + 1) * P, :])

            # v holds, per head, 64 value columns + 1 ones column (65 each)
            v = [pp.tile([P, NH * (D + 1)], BF16, tag=f"v{i}", name=f"v{i}")
                 for i in range(KC)]

            # ---- V projection, emitted lazily inside the fc=0 attention
            # stream (after qc=0's scores) so PE starts on K/Q projections
            # the moment xt lands while wv streams in on the scalar queue ----
            wv_sb = [pp.tile([P, H], BF16, tag=f"wv{k}", name=f"wvt{k}")
                     for k in range(KC)]
            for k in range(KC):
                # scalar (ACT) hwdge queue: parallel with the xt loads
                # on sync, and ACT is idle until the first exp anyway
                nc.scalar.dma_start(wv_sb[k][:], wvT.ap()[k * P:(k + 1) * P, :])

            def emit_v_projection():
                for sc in range(KC):
                    vv = v[sc].rearrange("p (h e) -> p h e", e=D + 1)
                    nc.vector.tensor_copy(
                        vv[:, :, D:D + 1], ones16[:].unsqueeze(2))
                    for fn in range(QC):
                        ps = psA.tile([P, NQ], F32, tag="mm")
                        for k in range(KC):
                            nc.tensor.matmul(
                                ps[:],
                                xt[k][:, sc * P:(sc + 1) * P],
                                wv_sb[k][:, fn * NQ:(fn + 1) * NQ],
                                start=(k == 0), stop=(k == KC - 1),
                            )
                        nc.vector.tensor_add(
                            vv[:, fn * 8:(fn + 1) * 8, 0:D],
                            ps[:].rearrange("p (h d) -> p h d", d=D),
                            bvb.rearrange("p (h d) -> p h d", d=D)[:, fn * 8:(fn + 1) * 8, :],
                        )
                    # fold the attention mask in: scale key-row sc*128+p of
                    # every head's values AND its ones column by exp(mask)
                    nc.vector.tensor_scalar_mul(
                        v[sc][:], v[sc][:], em_sb[:, sc:sc + 1])

            # ---- streamed per-fc: K/Q projection then attention ----
            with (
                tc.tile_pool(name="wfp", bufs=2) as wfp,
                tc.tile_pool(name="qkp", bufs=2) as qkp,
                tc.tile_pool(name="ep", bufs=16) as ep,
                tc.tile_pool(name="misc", bufs=2) as mp,
            ):
                ors_by_qc = {
                    qc: [pp.tile([P, H], F32, tag=f"or_{qc}_{j}",
                                 name=f"or_{qc}_{j}") for j in range(4)]
                    for qc in range(QC)
                }

                def flush(pr):
                    qcp, fcp, base, pvs = pr
                    # both PSUM->SBUF copies first: the tp tiles below share
                    # the "pv" tag slots, so slot reuse must wait only on
                    # these copies (emitting cth1 after tp1 would deadlock)
                    cths = []
                    for hh in range(2):
                        cth = mp.tile([D + 1, NQ], BF16, tag=f"ct{hh}",
                                      name=f"ct_{qcp}_{fcp}_{hh}")
                        nc.vector.tensor_copy(cth[:], pvs[hh][:])
                        cths.append(cth)
                    for hh in range(2):
                        h = 2 * fcp + hh
                        cth = cths[hh]
                        for jq in range(4):
                            tp = psB.tile([P, D + 1], BF16, tag="pv",
                                          name=f"tp_{qcp}_{fcp}_{hh}_{jq}")
                            nc.tensor.transpose(
                                tp[:], cth[:, jq * P:(jq + 1) * P],
                                ident[0:D + 1, 0:D + 1])
                            rc = mp.tile([P, 1], F32, tag="rc",
                                         name=f"rc_{qcp}_{fcp}_{hh}_{jq}")
                            nc.vector.reciprocal(rc[:], tp[:, D:D + 1])
                            nc.vector.tensor_scalar_mul(
                                ors_by_qc[qcp][jq][:, h * D:(h + 1) * D],
                                tp[:, 0:D], rc[:])
                    if fcp == FC - 1:
                        # full 4KB-row stores: descriptor-friendly on the
                        # sync hwdge queue; in the steady-state loop these
                        # tail stores overlap the next iteration's head
                        for jq in range(4):
                            nc.sync.dma_start(
                                out.ap()[qcp * NQ + jq * P:
                                         qcp * NQ + (jq + 1) * P, :],
                                ors_by_qc[qcp][jq][:])

                # ---- score segment stream: each [P, 512] score block is a
                # segment; one [P, 1536] PSUM tile holds 3 segments and one
                # exp instruction covers all three (amortizing the ~352-cycle
                # ACT per-instruction overhead). Tiles deliberately span
                # k-chunk / pair / fc boundaries.
                TSEG = 3
                es_segs = {}            # global segment -> (es tile, offset)
                st = {"seg": 0, "ps": None}

                def emit_exp_tile(n):
                    s0 = st["seg"] - n
                    e = ep.tile([P, TSEG * NQ], BF16, tag="e",
                                name=f"e{s0}")
                    nc.scalar.activation(
                        e[:, 0:n * NQ], st["ps"][:, 0:n * NQ],
                        mybir.ActivationFunctionType.Exp, scale=0.125,
                    )
                    for j in range(n):
                        es_segs[s0 + j] = (e, (j % TSEG) * NQ)

                def seg_mm(kt, qt, qc, k, hh):
                    s = st["seg"]
                    j = s % TSEG
                    if j == 0:
                        st["ps"] = psA.tile([P, TSEG * NQ], F32, tag="mm",
                                            name=f"ps{s}")
                    lo, hi = hh * D, (hh + 1) * D
                    nc.tensor.matmul(
                        st["ps"][:, j * NQ:(j + 1) * NQ],
                        kt[lo:hi, k * P:(k + 1) * P],
                        qt[lo:hi, qc * NQ:(qc + 1) * NQ],
                        start=True, stop=True,
                        tile_position=(hh * D, 0),
                    )
                    st["seg"] = s + 1
                    if j == TSEG - 1:
                        emit_exp_tile(TSEG)

                def pv_es(base, k, hh):
                    e, off = es_segs[base + 2 * k + hh]
                    return e[:, off:off + NQ]

                prev = None
                for fc in range(FC):
                    # K and Q projections for this feature chunk; host
                    # rearrangement makes block (fc, k) one contiguous read
                    wts = {}
                    for nm, wR in (("k", wkR), ("q", wqR)):
                        wts[nm] = [
                            wfp.tile([P, P], BF16, tag=f"w{nm}{k}",
                                     name=f"w{nm}_{fc}_{k}")
                            for k in range(KC)
                        ]
                        for k in range(KC):
                            blk = fc * KC + k
                            nc.sync.dma_start(
                                wts[nm][k][:],
                                wR.ap()[blk * P:(blk + 1) * P, :])
                    kt = qkp.tile([P, S], BF16, tag="kt", name=f"kt{fc}")
                    qt = qkp.tile([P, S], BF16, tag="qt", name=f"qt{fc}")
                    for nm, b_sb, dst in (("k", bk_sb, kt), ("q", bq_sb, qt)):
                        for sc in range(QC):
                            ps = psA.tile([P, NQ], F32, tag="mm")
                            for k in range(KC):
                                nc.tensor.matmul(
                                    ps[:],
                                    wts[nm][k][:],
                                    xt[k][:, sc * NQ:(sc + 1) * NQ],
                                    start=(k == 0), stop=(k == KC - 1),
                                )
                            nc.vector.tensor_scalar_add(
                                dst[:, sc * NQ:(sc + 1) * NQ], ps[:],
                                b_sb[:, fc:fc + 1],
                            )

                    # attention for this fc's two heads, pipelined: pair
                    # i's PV matmuls interleave with pair i+1's scores/exp
                    for qc in range(QC):
                        if fc == 0 and qc == 1:
                            # v must be ready before the first PV matmuls
                            # (pair (0,0), issued inside this qc's k-loop);
                            # ACT exps pair (0,0) under these 128 matmuls
                            emit_v_projection()
                        base = st["seg"]
                        for k in range(KC):
                            for hh in range(2):
                                seg_mm(kt, qt, qc, k, hh)
                            if prev is not None:
                                qcp, fcp, basep, pvs_p = prev
                                if k == 0:
                                    pvs_p = (
                                        psB.tile([D + 1, NQ], F32, tag="pv",
                                                 name=f"pv0_{qcp}_{fcp}"),
                                        psB.tile([D + 1, NQ], F32, tag="pv",
                                                 name=f"pv1_{qcp}_{fcp}"),
                                    )
                                    prev = (qcp, fcp, basep, pvs_p)
                                for hh in range(2):
                                    h = 2 * fcp + hh
                                    nc.tensor.matmul(
                                        pvs_p[hh][:],
                                        v[k][:, h * (D + 1):(h + 1) * (D + 1)],
                                        pv_es(basep, k, hh),
                                        start=(k == 0), stop=(k == KC - 1),
                                    )
                        if prev is not None:
                            flush(prev)
                        prev = (qc, fc, base, None)
                # drain: finish any partial exp tile, then the last pair
                if st["seg"] % TSEG != 0:
                    emit_exp_tile(st["seg"] % TSEG)
                qcp, fcp, basep, _ = prev
                pvs_p = (
                    psB.tile([D + 1, NQ], F32, tag="pv", name="pv0_last"),
                    psB.tile([D + 1, NQ], F32, tag="pv", name="pv1_last"),
                )
                for k in range(KC):
                    for hh in range(2):
                        h = 2 * fcp + hh
                        nc.tensor.matmul(
                            pvs_p[hh][:],
                            v[k][:, h * (D + 1):(h + 1) * (D + 1)],
                            pv_es(basep, k, hh),
                            start=(k == 0), stop=(k == KC - 1),
                        )
                flush((qcp, fcp, basep, pvs_p))

    nc.compile()
    return nc


def _get_nc(loop: int = 1):
    key = ("nc", loop)
    if key not in _CACHE:
        _CACHE[key] = _build(loop)
    return _CACHE[key]


def _bf16(a: np.ndarray) -> np.ndarray:
    import ml_dtypes

    return np.ascontiguousarray(a.astype(ml_dtypes.bfloat16))


def _rearrange_w(wT: np.ndarray) -> np.ndarray:
    """[H, H] wT -> [FC*KC*P, P] where block (fc, k) = wT[k-rows, fc-cols],
    so each per-fc weight slice DMA is one contiguous read."""
    return np.ascontiguousarray(
        wT.reshape(KC, P, FC, P).transpose(2, 0, 1, 3).reshape(FC * KC * P, P)
    )


def prep_inputs(inputs: dict) -> list[dict]:
    hs = np.ascontiguousarray(np.asarray(inputs["hidden_states"], dtype=np.float32))
    am = np.asarray(inputs["attention_mask"], dtype=np.float32)
    wq = np.asarray(inputs["Wq"], dtype=np.float32)
    wk = np.asarray(inputs["Wk"], dtype=np.float32)
    wv = np.asarray(inputs["Wv"], dtype=np.float32)
    bq = np.ascontiguousarray(np.asarray(inputs["bq"], dtype=np.float32))
    bk = np.ascontiguousarray(np.asarray(inputs["bk"], dtype=np.float32))
    bv = np.ascontiguousarray(np.asarray(inputs["bv"], dtype=np.float32))

    n_cores = 8
    assert hs.shape == (n_cores, S, H)
    wqR = _bf16(_rearrange_w(wq.T))
    wkR = _bf16(_rearrange_w(wk.T))
    wvT = _bf16(wv.T)
    am = np.broadcast_to(am, (n_cores, 1, 1, S))

    in_maps = []
    for b in range(n_cores):
        in_maps.append({
            "xT": _bf16(hs[b].T),
            "wqR": wqR, "wkR": wkR, "wvT": wvT,
            "bq": bq, "bk": bk, "bv": bv,
            "mask": np.ascontiguousarray(am[b, 0, 0, :]),
        })
    return in_maps


def kernel(**inputs) -> np.ndarray:
    n_cores = 8
    in_maps = prep_inputs(inputs)
    nc = _get_nc()
    try:
        res = run_bass_kernel_spmd(nc, in_maps, core_ids=list(range(n_cores)))
    except Exception:
        # the shared axon terminal occasionally throws a transient
        # NRT_EXEC_UNIT_UNRECOVERABLE; one retry usually succeeds
        time.sleep(5)
        res = run_bass_kernel_spmd(nc, in_maps, core_ids=list(range(n_cores)))
    return np.stack([res.results[b]["out"] for b in range(n_cores)], axis=0)


if __name__ == "__main__":
    rng = np.random.default_rng(0)
    ins = {
        "hidden_states": rng.standard_normal((8, S, H), dtype=np.float32),
        "attention_mask": np.zeros((8, 1, 1, S), np.float32),
        "Wq": rng.standard_normal((H, H), dtype=np.float32) / 32,
        "bq": rng.standard_normal(H, dtype=np.float32) * 0.1,
        "Wk": rng.standard_normal((H, H), dtype=np.float32) / 32,
        "bk": rng.standard_normal(H, dtype=np.float32) * 0.1,
        "Wv": rng.standard_normal((H, H), dtype=np.float32) / 32,
        "bv": rng.standard_normal(H, dtype=np.float32) * 0.1,
    }
    got = kernel(**ins)
    print("out", got.shape, got.dtype, float(np.abs(got).mean()))
